# revision 58
# baseline (speedup 1.0000x reference)
"""HAN (hypergraph attention network) Trainium2 kernel, v4 (fp8).

Data-parallel over batch: 8 cores x 16 batch elements each, all params
replicated. Pipeline per core:
  - Per-core vocabulary compaction on host (<= 13056 unique tokens) so
    embedding-gather indices fit int16, table stored fp8e4m3 x8 padded
    to 512 B rows.
  - Transposing InstDMAGatherAnt ('mlp' gpsimd library) lands gathered
    rows feature-major in SBUF; <=768 idxs per gather (HW limit), one
    gather per batch element.
  - Projections as fp8 DoubleRow matmuls (2 k-rows/cycle); biases ride
    a hijacked constant pad feature so PSUM drains are pure copies
    batching two c-chunks; all fp8 scales folded into the exp and
    fc-out activations (scale=1/4096) -- zero extra ops.
  - Bilinear attention: logits via X^T = hq*h2att DVE product, softmax
    with accum-exp (max-sub skipped: logits are tiny), per-head bilinear
    readout via token-major hs (single-bank f16 PE transposes).
  - fc weights + glove candidates prefetched mid-loop (write-dep gates
    defeat DMA hoisting ahead of the critical prologue loads); sim +
    log_softmax epilogue with the final subtract split across DVE/Act.
"""

import numpy as np
import ml_dtypes
from contextlib import ExitStack

import concourse.bass as bass
import concourse.bacc as bacc
import concourse.tile as tile
from concourse import library_config, mybir
from concourse.bass_utils import run_bass_kernel_spmd

F32 = mybir.dt.float32
F16 = mybir.dt.float16
F8 = mybir.dt.float8e4
I16 = mybir.dt.int16
AF = mybir.ActivationFunctionType
ALU = mybir.AluOpType
AX = mybir.AxisListType
PM = mybir.MatmulPerfMode

# Problem shapes (hardcoded per contract)
NCORES = 8
B = 128
BPC = B // NCORES          # 16 batch elems per core
NQ, NS, NODES = 16, 256, 3
V, E = 50000, 300
EP = 512                   # fp8 emb row padded to 512 B (DMA 256B rule)
NU = 16384                 # compacted per-core vocab rows (>= max unique)
ESC = 8.0                  # fp8 range scale on emb AND weights (out /64)
C, H, OUT, NA = 1024, 8, 300, 5000
CC = C // 128              # 8 c-chunks
NCH = 6                    # DoubleRow chunks: (node j, 256-feat half)
OCN = [128, 128, 44]       # OUT=300 -> 3 o-chunks
SIMCH = [512] * 9 + [392]  # NA=5000 N-chunks
NKG = 2 * NS * NODES       # 1536 gather idxs per pair
NQG = BPC * NQ * NODES     # 768 gather idxs for all ques

_CACHED = None


def _emit(ctx, tc, ins, outs):
    nc = tc.nc

    embc = ins["embc"]          # [NU, EP] f8 (per-core compacted, x8)
    idx_d = ins["idx"]          # [128, 816] i16
    kwT_d = ins["kwT"]          # [128, NCH*2*1024] f8 (x8)
    qwT_d = ins["qwT"]
    h2aT_d = ins["h2aT"]        # [128, CC*H] f32
    fcb_d = ins["fcb"]          # [128, 3] f32
    sel1_d = ins["sel1"]        # [128, H] f32
    ones1_d = ins["ones1"]      # [1, 128] f32
    negl_d = ins["negl"]        # [128, 1] f32 = -ln(4096)
    idhf_d = ins["idhf"]        # [128, 128] f16
    fcwT_d = ins["fcwT"]        # [128, H*CC*OUT] f16
    gloT_d = ins["gloT"]        # [128, 3*NA] f16 (col a*3+oc)
    out_d = outs["out"]         # [BPC, NA] f32

    const = ctx.enter_context(tc.tile_pool(name="const", bufs=1))
    katp = ctx.enter_context(tc.tile_pool(name="katp", bufs=3))
    hstp = ctx.enter_context(tc.tile_pool(name="hstp", bufs=3))
    hsbp = ctx.enter_context(tc.tile_pool(name="hsbp", bufs=3))
    xtp = ctx.enter_context(tc.tile_pool(name="xtp", bufs=3))
    attp = ctx.enter_context(tc.tile_pool(name="attp", bufs=3))
    tmpp = ctx.enter_context(tc.tile_pool(name="tmpp", bufs=3))
    smlp = ctx.enter_context(tc.tile_pool(name="smlp", bufs=3))

    pspj = ctx.enter_context(tc.tile_pool(name="pspj", bufs=3, space="PSUM"))
    pstr = ctx.enter_context(tc.tile_pool(name="pstr", bufs=2, space="PSUM"))
    psyt = ctx.enter_context(tc.tile_pool(name="psyt", bufs=3, space="PSUM"))

    # ---- resident weights/constants ----
    # Order matters: idx + qwT first (gate the prologue gather+project);
    # fcw (big, needed ~200us later) goes on the Act engine's HWDGE queue.
    idxT = const.tile([128, 816], I16, tag="idxT")
    nc.sync.dma_start(idxT[:], idx_d[:])
    qwT = const.tile([128, NCH * 2 * 1024], F8, tag="qwT")
    nc.sync.dma_start(qwT[:, 0:6144], qwT_d[:, 0:6144])
    nc.sync.dma_start(qwT[:, 6144:], qwT_d[:, 6144:])
    kwT = const.tile([128, NCH * 2 * 1024], F8, tag="kwT")
    h2aT = const.tile([128, CC * H], F32, tag="h2aT")
    nc.sync.dma_start(h2aT[:], h2aT_d[:])
    fcb = const.tile([128, 3], F32, tag="fcb")
    nc.sync.dma_start(fcb[:], fcb_d[:])
    sel1 = const.tile([128, H], F32, tag="sel1")
    nc.sync.dma_start(sel1[:], sel1_d[:])
    ones1 = const.tile([1, 128], F32, tag="ones1")
    nc.sync.dma_start(ones1[:], ones1_d[:])
    negl = const.tile([128, 1], F32, tag="negl")
    nc.sync.dma_start(negl[:], negl_d[:])
    idhf = const.tile([128, 128], F16, tag="idhf")
    nc.sync.dma_start(idhf[:], idhf_d[:])
    # fcw/glove are needed only in the epilogue; their loads are issued
    # inside the main loop (Act HWDGE queue) so they never contend with the
    # prologue's gather/weight loads.
    fcw = const.tile([128, H * CC * OUT], F16, tag="fcw")
    glo = const.tile([128, 3 * NA], F16, tag="glo")

    hqT = const.tile([128, CC * 256], F32, tag="hqT")      # [c, b*16+q]
    POOL = const.tile([128, CC * BPC * H], F16, tag="POOL")  # col cc*128+b*8+h
    fcout = const.tile([128, 3 * BPC], F16, tag="fcout")
    sim_sb = const.tile([BPC, NA], F32, tag="sim_sb")
    parti = const.tile([BPC, 16], F32, tag="parti")
    lse = const.tile([BPC, 1], F32, tag="lse")
    tot = const.tile([BPC, 1], F32, tag="tot")
    denr = const.tile([1, 128], F32, tag="denr")   # col b*8+h: denom/4096
    rden = const.tile([1, 128], F32, tag="rden")

    def project(wT, act, dst, dcol):
        """dst[:, dcol + cc*pitch : +256] = wT.T @ act  (value x ESC^2).

        fp8 DoubleRow: act is a 768-token transposed-gather tile
        [128, 3072] f8 with col = c2*1536 + jn*512 + 2s + j2 (feature
        c2*256 + 2p + j2 of node jn, token s). wT holds 6 chunks
        t=(jn*2+c2), each [128, 2(j2), 1024(c)], scaled by ESC; bias
        rides on the constant-1 pad feature (host-packed), so the drain
        is a pure copy batching two c-chunks per op."""
        av = act.rearrange("p (c jn s j) -> p c jn j s", c=2, jn=3, j=2)
        wv = wT[:].rearrange("p (t j m) -> p t j m", t=NCH, j=2)
        dv = dst[:].rearrange("p (c t) -> p c t", c=CC)
        for cc2 in range(CC // 2):
            ps = pspj.tile([128, 512], F32, tag="pjps")
            for half in range(2):
                cc = cc2 * 2 + half
                for t in range(NCH):
                    jn, c2 = divmod(t, 2)
                    nc.tensor.matmul(
                        out=ps[:, half * 256: half * 256 + 256],
                        lhsT=wv[:, t, :, cc * 128:(cc + 1) * 128],
                        rhs=av[:, c2, jn],
                        start=(t == 0),
                        stop=(t == NCH - 1),
                        perf_mode=PM.DoubleRow,
                    )
            nc.scalar.copy(
                out=dv[:, cc2 * 2: cc2 * 2 + 2, dcol: dcol + 256],
                in_=ps[:].rearrange("p (c t) -> p c t", c=2),
            )

    # InstDMAGatherAnt lives in the 'mlp' gpsimd library; load it before
    # the first gather (missing load hangs the Q7 cores on hardware).
    nc.gpsimd.load_library(library_config.mlp)

    # ---- prologue: gather+project hq for all 16 b (256 ques tokens) ----
    qact = const.tile([128, 4 * NQG], F8, tag="qact")
    nc.gpsimd.dma_gather(
        qact[:].rearrange("p (a i) -> p a i", a=4),
        embc[:],
        idxT[:, 768:816],
        NQG, NQG, EP,
        transpose=True,
    )
    nc.vector.tensor_copy(kwT[0:1, 0:1], qact[0:1, 0:1])
    nc.sync.dma_start(kwT[:, 0:6144], kwT_d[:, 0:6144])
    nc.sync.dma_start(kwT[:, 6144:], kwT_d[:, 6144:])
    project(qwT, qact[:], hqT, 0)

    hqv = hqT[:].rearrange("p (c t) -> p c t", c=CC)  # [128, 8, 256]
    h2av = h2aT[:].rearrange("p (c h) -> p c h", c=CC)  # [128, 8, 8]

    pv = POOL[:].rearrange("p (c b h) -> p c b h", c=CC, b=BPC)

    # ---- per pair of batch elements ----
    for bp in range(BPC // 2):
        if bp == 1:
            nc.scalar.copy(out=fcw[0:1, 0:1], in_=hqT[0:1, 0:1])
            nc.scalar.dma_start(fcw[:], fcwT_d[:])
        elif bp == 3:
            nc.scalar.copy(out=glo[0:1, 0:1], in_=hqT[0:1, 0:1])
            nc.scalar.dma_start(glo[:, 0: 3 * 2500], gloT_d[:, 0: 3 * 2500])
        elif bp == 5:
            nc.scalar.copy(out=glo[0:1, 7500:7501], in_=hqT[0:1, 0:1])
            nc.scalar.dma_start(glo[:, 3 * 2500:], gloT_d[:, 3 * 2500:])
        # per-b gathers of 768 idxs (the HW gather wedges above ~768) and
        # per-b fp8 projections into each half of the pair's hsT
        kact = katp.tile([128, 2 * 4 * (NKG // 2)], F8, tag="kact")
        kav = kact[:].rearrange("p (g a i) -> p g a i", g=2, a=4)
        hsT = hstp.tile([128, CC * 512], F16, tag="hsT")
        for g in range(2):
            b = 2 * bp + g
            nc.gpsimd.dma_gather(
                kav[:, g],
                embc[:],
                idxT[:, b * 48:(b + 1) * 48],
                NKG // 2, NKG // 2, EP,
                transpose=True,
            )
            project(kwT, kact[:, g * 3072:(g + 1) * 3072], hsT, g * 256)

        for g in range(2):
            b = bp * 2 + g
            hb = g * 256  # this b's token offset inside the pair

            # hs token-major f16: [s-part, col st*1024 + cc*128 + c]
            hs_sb = hsbp.tile([128, 2 * 1024], F16, tag="hs_sb")
            for st in range(2):
                ps = pstr.tile([128, 1024], F16, tag="trps")
                for cc in range(8):
                    nc.tensor.transpose(
                        out=ps[:, cc * 128:(cc + 1) * 128],
                        in_=hsT[:, cc * 512 + hb + st * 128:
                                cc * 512 + hb + st * 128 + 128],
                        identity=idhf[:],
                    )
                if st == 0:
                    nc.scalar.copy(
                        out=hs_sb[:, 0:1024], in_=ps[:])
                else:
                    nc.vector.tensor_copy(
                        hs_sb[:, 1024:2048], ps[:])

            # X^T[c, h*16+q] = hqT[c, q] * h2aT[c, h]  (one grouped DVE op)
            XT = xtp.tile([128, 1024], F16, tag="XT")
            nc.vector.tensor_tensor(
                out=XT[:].rearrange("p (c h q) -> p c h q", c=CC, h=H),
                in0=hqv[:, :, b * 16: b * 16 + 16].unsqueeze(2).to_broadcast(
                    [128, CC, H, 16]),
                in1=h2av[:, :, :].unsqueeze(3).to_broadcast([128, CC, H, 16]),
                op=ALU.mult,
            )

            # logits[hq=128, s=256]
            plg = pspj.tile([128, 512], F32, tag="pjps")
            for cc in range(CC):
                nc.tensor.matmul(
                    out=plg[:, 0:256],
                    lhsT=XT[:, cc * 128: cc * 128 + 128],
                    rhs=hsT[:, cc * 512 + hb: cc * 512 + hb + 256],
                    start=(cc == 0),
                    stop=(cc == CC - 1),
                )

            # softmax numerator only: att = exp(logits - ln 4096) (f16-safe
            # scale); the per-(b,h) denominator is deferred to a per-pair
            # POOL normalization, shortening the exp->YT critical chain.
            att = attp.tile([128, 256], F16, tag="att")
            qsum = smlp.tile([128, 1], F32, tag="qsum")
            nc.scalar.activation(att[:], plg[:, 0:256], AF.Exp,
                                 scale=1.0 / ESC ** 4, bias=negl[:],
                                 accum_out=qsum[:])

            # denom row [1, 8] for this b (off the critical path)
            dps = psyt.tile([128, 512], F32, tag="ytps", name="dps")
            nc.tensor.matmul(out=dps[0:1, 0:8], lhsT=qsum[:], rhs=sel1[:],
                             start=True, stop=True)
            nc.vector.tensor_copy(denr[0:1, b * 8:(b + 1) * 8],
                                  dps[0:1, 0:8])

            # attT [s, hq] f16
            attT = attp.tile([128, 256], F16, tag="attT")
            psTb = pstr.tile([128, 256], F16, tag="trps")
            for st in range(2):
                nc.tensor.transpose(
                    out=psTb[:, st * 128:(st + 1) * 128],
                    in_=att[:, st * 128:(st + 1) * 128],
                    identity=idhf[:],
                )
            nc.vector.tensor_copy(attT[:], psTb[:])

            # YT[c, hq] per c-chunk; pooled[h,c] = sum_q hqT * sum_s attT*hs
            for ccg in range(2):
                py = psyt.tile([128, 512], F32, tag="ytps")
                for i in range(4):
                    cc = ccg * 4 + i
                    for st in range(2):
                        nc.tensor.matmul(
                            out=py[:, i * 128:(i + 1) * 128],
                            lhsT=hs_sb[:, st * 1024 + cc * 128:
                                       st * 1024 + cc * 128 + 128],
                            rhs=attT[:, st * 128:(st + 1) * 128],
                            start=(st == 0),
                            stop=(st == 1),
                        )
                tmp = tmpp.tile([128, 512], F32, tag="tmp")
                nc.vector.tensor_tensor(
                    out=tmp[:].rearrange("p (c h q) -> p c h q", c=4, h=H),
                    in0=py[:].rearrange("p (c h q) -> p c h q", c=4, h=H),
                    in1=hqv[:, ccg * 4:(ccg + 1) * 4,
                            b * 16: b * 16 + 16].unsqueeze(2).to_broadcast(
                                [128, 4, H, 16]),
                    op=ALU.mult,
                )
                with nc.allow_low_precision(reason="16-elem q-sum, tiny"):
                    nc.vector.reduce_sum(
                        out=pv[:, ccg * 4:(ccg + 1) * 4, b, :],
                        in_=tmp[:].rearrange("p (c h q) -> p c h q", c=4, h=H),
                        axis=AX.X,
                    )

        # normalize this pair's POOL slice by 1/denom (rank-1 broadcast)
        nc.vector.reciprocal(rden[0:1, bp * 16:(bp + 1) * 16],
                             denr[0:1, bp * 16:(bp + 1) * 16])
        prb = psyt.tile([128, 512], F32, tag="ytps", name="prb")
        nc.tensor.matmul(out=prb[:, 0:16], lhsT=ones1[:],
                         rhs=rden[0:1, bp * 16:(bp + 1) * 16],
                         start=True, stop=True)
        nc.vector.tensor_tensor(
            out=pv[:, :, 2 * bp: 2 * bp + 2, :],
            in0=pv[:, :, 2 * bp: 2 * bp + 2, :],
            in1=prb[:, 0:16].rearrange(
                "p (b h) -> p b h", b=2).unsqueeze(1).to_broadcast(
                    [128, CC, 2, H]),
            op=ALU.mult,
        )

    # ---- fc: out[o, b] = sum_{h,c} fc_w[o, h*1024+c] * pooled ----
    poolv = POOL[:].rearrange("p (c b h) -> p c b h", c=CC, b=BPC)
    # 3 accumulators in 3 different PSUM banks (concurrent open groups
    # in one bank are illegal); pools are otherwise idle in this phase.
    pfc = [pspj.tile([128, 512], F32, tag="pjps", name="pfc0"),
           pstr.tile([128, 512], F32, tag="trps", name="pfc1"),
           psyt.tile([128, 512], F32, tag="ytps", name="pfc2")]
    nhc = H * CC
    for h in range(H):
        for cc in range(CC):
            i = h * CC + cc
            for oc in range(3):
                ocn = OCN[oc]
                nc.tensor.matmul(
                    out=pfc[oc][0:ocn, 0:16],
                    lhsT=fcw[:, i * OUT + oc * 128: i * OUT + oc * 128 + ocn],
                    rhs=poolv[:, cc, :, h],
                    start=(i == 0),
                    stop=(i == nhc - 1),
                )
    for oc in range(3):
        ocn = OCN[oc]
        nc.scalar.activation(
            out=fcout[0:ocn, oc * 16: oc * 16 + 16],
            in_=pfc[oc][0:ocn, 0:16],
            func=AF.Identity,
            bias=fcb[0:ocn, oc: oc + 1],
            scale=1.0 / ESC ** 4,
        )

    # ---- sim = fcout.T @ gloveT ; log_softmax over NA ----
    glov = glo[:].rearrange("p (a o) -> p a o", o=3)
    a0 = 0
    for ci, n in enumerate(SIMCH):
        pss = psyt.tile([16, 512], F32, tag="ytps", name="pss")
        for oc in range(3):
            ocn = OCN[oc]
            nc.tensor.matmul(
                out=pss[0:16, 0:n],
                lhsT=fcout[0:ocn, oc * 16: oc * 16 + 16],
                rhs=glov[0:ocn, a0: a0 + n, oc],
                start=(oc == 0),
                stop=(oc == 2),
            )
        junk = tmpp.tile([128, 512], F32, tag="tmp")
        nc.scalar.activation(junk[0:16, 0:n], pss[0:16, 0:n], AF.Exp,
                             accum_out=parti[:, ci: ci + 1])
        nc.vector.tensor_copy(sim_sb[:, a0: a0 + n], pss[0:16, 0:n])
        a0 += n

    nc.vector.reduce_sum(out=tot[:], in_=parti[:, 0:10], axis=AX.X)
    nc.scalar.activation(lse[:], tot[:], AF.Ln)
    nlse = smlp.tile([BPC, 1], F32, tag="nlse")
    nc.vector.tensor_scalar_mul(nlse[:], lse[:], -1.0)
    for qt in range(4):
        c0, c1 = qt * 1250, (qt + 1) * 1250
        if qt % 2 == 0:
            nc.vector.tensor_scalar_sub(sim_sb[:, c0:c1],
                                        sim_sb[:, c0:c1], lse[:])
        else:
            nc.scalar.activation(out=sim_sb[:, c0:c1], in_=sim_sb[:, c0:c1],
                                 func=AF.Identity, bias=nlse[:])
        nc.sync.dma_start(out_d[:, c0:c1], sim_sb[:, c0:c1])


def _build():
    nc = bacc.Bacc("TRN2", target_bir_lowering=False, debug=False,
                   num_devices=NCORES)
    ins = {}

    def di(name, shape, dtype):
        ins[name] = nc.dram_tensor(name, list(shape), dtype,
                                   kind="ExternalInput").ap()

    di("embc", (NU, EP), F8)
    di("idx", (128, 816), I16)
    di("kwT", (128, NCH * 2 * 1024), F8)
    di("qwT", (128, NCH * 2 * 1024), F8)
    di("h2aT", (128, CC * H), F32)
    di("fcb", (128, 3), F32)
    di("sel1", (128, H), F32)
    di("ones1", (1, 128), F32)
    di("negl", (128, 1), F32)
    di("idhf", (128, 128), F16)
    di("fcwT", (128, H * CC * OUT), F16)
    di("gloT", (128, 3 * NA), F16)
    outs = {"out": nc.dram_tensor("out", [BPC, NA], F32,
                                  kind="ExternalOutput").ap()}

    with tile.TileContext(nc) as tc, ExitStack() as ctx:
        _emit(ctx, tc, ins, outs)
    nc.compile()
    return nc


def _pack_host(q2h_w, q2h_b, k2h_w, k2h_b, h2att_w, fc_w, fc_b,
               glove_cands):
    """One-time layout prep of replicated params (host numpy)."""
    f32 = np.float32
    f16 = np.float16

    f8 = np.dtype(mybir.dt.np(mybir.dt.float8e4))

    def packT(W, b):
        # W [C, 900] -> [128, NCH*2*1024] f8 (x ESC): col
        # ((jn*2+c2)*2 + j2)*1024 + c holds W[c, jn*300 + c2*256 + 2p + j2].
        # The bias rides on node 0's constant pad feature E (emb stores ESC
        # there), so psum = ESC^2 * (W @ x + b) with no drain-side bias.
        Wp = np.zeros((C, NODES, EP), f32)
        Wp[:, :, :E] = np.asarray(W, f32).reshape(C, NODES, E) * ESC
        Wp[:, 0, E] = np.asarray(b, f32) * ESC
        return np.ascontiguousarray(
            Wp.reshape(C, NODES, 2, 128, 2).transpose(3, 1, 2, 4, 0)
            .reshape(128, NCH * 2 * C)).astype(f8)

    kwT = packT(k2h_w, k2h_b)
    qwT = packT(q2h_w, q2h_b)

    h2aT = np.zeros((128, CC * H), f32)
    for cc in range(CC):
        h2aT[:, cc * H:(cc + 1) * H] = \
            np.asarray(h2att_w, f32)[:, cc * 128:(cc + 1) * 128].T

    fcb = np.zeros((128, 3), f32)
    fcb_src = np.asarray(fc_b, f32)
    for oc in range(3):
        fcb[0:OCN[oc], oc] = fcb_src[oc * 128: oc * 128 + OCN[oc]]

    sel1 = np.zeros((128, H), f32)
    for p in range(128):
        sel1[p, p // 16] = 1.0
    ones1 = np.ones((1, 128), f32)
    negl = np.full((128, 1), -np.log(ESC ** 4), f32)

    idhf = np.eye(128, dtype=f16)

    # fc_w [OUT, H*C]: col (h*CC+cc)*OUT + o = fc_w[o, h*1024+cc*128+p]
    fcw = np.asarray(fc_w, f32).reshape(OUT, H, CC, 128)
    fcwT = np.ascontiguousarray(
        fcw.transpose(3, 1, 2, 0).reshape(128, H * CC * OUT)).astype(f16)

    # glove [NA, OUT] -> [128, NA*3]: col a*3+oc = glove[a, oc*128+p]
    glo = np.asarray(glove_cands, f32)
    G = np.zeros((3, 128, NA), f32)
    for oc in range(3):
        G[oc, 0:OCN[oc], :] = glo[:, oc * 128: oc * 128 + OCN[oc]].T
    gloT = np.ascontiguousarray(
        G.transpose(1, 2, 0).reshape(128, NA * 3)).astype(f16)

    return dict(kwT=kwT, qwT=qwT, h2aT=h2aT, fcb=fcb,
                sel1=sel1, ones1=ones1, negl=negl, idhf=idhf,
                fcwT=fcwT, gloT=gloT)


_PACK_CACHE = {}


def _key(*arrs):
    h = 0
    for a in arrs:
        a = np.asarray(a)
        h ^= hash((a.shape, a.dtype.str,
                   a.reshape(-1)[:: max(1, a.size // 64)].tobytes()))
    return h


def make_in_maps(he_ques, he_kg, emb, q2h_w, q2h_b, k2h_w, k2h_b,
                 h2att_w, h2att_b, fc_w, fc_b, glove_cands):
    pk = _key(q2h_w, k2h_w, fc_w, glove_cands)
    if pk not in _PACK_CACHE:
        _PACK_CACHE.clear()
        _PACK_CACHE[pk] = _pack_host(q2h_w, q2h_b, k2h_w, k2h_b,
                                     h2att_w, fc_w, fc_b, glove_cands)
        f8 = np.dtype(mybir.dt.np(mybir.dt.float8e4))
        _PACK_CACHE["emb8"] = (
            np.asarray(emb, np.float32) * ESC).astype(f8)
    shared = _PACK_CACHE[pk]
    emb8 = _PACK_CACHE["emb8"]

    he_kg = np.asarray(he_kg).astype(np.int64)
    he_ques = np.asarray(he_ques).astype(np.int64)
    nkg = BPC * NS * NODES
    maps = []
    for c in range(NCORES):
        kg_c = he_kg[c * BPC:(c + 1) * BPC]       # [16, 256, 3]
        q_c = he_ques[c * BPC:(c + 1) * BPC]      # [16, 16, 3]
        toks = np.concatenate([kg_c.reshape(-1), q_c.reshape(-1)])
        uniq, inv = np.unique(toks, return_inverse=True)
        assert len(uniq) <= NU
        embc = np.zeros((NU, EP), emb8.dtype)
        embc[:len(uniq), :E] = emb8[uniq]
        embc[:, E] = emb8.dtype.type(ESC)

        inv_kg = inv[:nkg].reshape(BPC, NS, NODES)
        inv_q = inv[nkg:].reshape(BPC, NQ, NODES)
        # per-b idx order: for j: for s: -> [16, 768]; idx i -> [i%16, i//16]
        b_idx = inv_kg.transpose(0, 2, 1).reshape(BPC, NS * NODES)
        kg_tiles = b_idx.reshape(BPC, NS * NODES // 16, 16).transpose(0, 2, 1)
        kg_cols = kg_tiles.transpose(1, 0, 2).reshape(16, BPC * NS * NODES // 16)
        q_flat = inv_q.transpose(2, 0, 1).reshape(NQG)
        q_tile = q_flat.reshape(NQG // 16, 16).T
        idx16 = np.concatenate([kg_cols, q_tile], axis=1)  # [16, 816]
        idx128 = np.ascontiguousarray(
            np.tile(idx16, (8, 1))).astype(np.int16)

        m = dict(shared)
        m["embc"] = embc
        m["idx"] = idx128
        maps.append(m)
    return maps


def kernel(**inputs):
    global _CACHED
    if _CACHED is None:
        _CACHED = _build()
    nc = _CACHED
    in_maps = make_in_maps(**inputs)
    res = run_bass_kernel_spmd(nc, in_maps, list(range(NCORES)))
    return np.concatenate([r["out"] for r in res.results], axis=0)
